# revision 12
# baseline (speedup 1.0000x reference)
"""Trainium2 Bass kernel for nn_CrossAttention (dense_transformer).  v3

Sharding: 8 cores = (batch 0..3) x (image half 0..1), 64 rows + 1 halo row
each side per core.  Only per-head Gram matrices and l2-norm square sums
are AllGather'd between the two cores of a batch.

v3 structure (vs v2, from trace analysis: DVE 69%/PE 65%/ACT 56% busy):
  - conv1x1 outputs land in a 130-pitch padded SBUF layout A (2 zero pad
    cols between rows, row base 4B-aligned), conv psum f32 N=512.
  - depthwise 9 taps split: 2 taps (q,k) / 4 taps (v) on PE as chained
    diagonal matmuls into bf16 psum (alignment-free); the ACT psum->acc
    copy doubles as the accumulator init; remaining taps are single-pass
    DVE scalar_tensor_tensor (2x mode): dx==0 taps read A at even offsets,
    odd taps read S = A<<1 (built by DMA, SBUF->SBUF) at even offsets.
  - acc (dw output) is padless pitch-128: flat DVE out APs, flat norms,
    flat proj windows, and batched DMA-xbar transposes ([128, 1024] ->
    [128, 8, 128] per-block) straight from acc -- no PE transposes.
  - Gram accumulated per spatial row over transposed q/k tiles (64 rows
    x 4 pair-blocks); q,k processed in interleaved 32-row half-tensors.
  - attention is folded into Wproj: M^T = A_head^T-weighted WprojT built
    on-PE after softmax; final conv1x1 uses M^T directly on the stored
    depthwise-v (attn@v matmuls and their copies disappear).
"""
import numpy as np
import ml_dtypes

import concourse.bass as bass
import concourse.bacc as bacc
from concourse import mybir
from concourse.bass_utils import run_bass_kernel_spmd
from concourse.tile import TileContext

F32 = mybir.dt.float32
BF16 = mybir.dt.bfloat16
ALU = mybir.AluOpType
AF = mybir.ActivationFunctionType

C = 384
HEADS = 8
W = 128
HALF = 64
HR = 32                  # output rows per half-tensor pass
AR = HR + 2              # conv rows per half (1 halo each side)
PITCH = 130              # A row pitch: [pad, pad, 128 data]
ASZ = 2 + AR * PITCH + 6   # 4428; leading/trailing zero guards
ACC = HR * W             # 4096, padless acc per half

# tap t: dy = t//3-1, dx = t%3-1; base offset into A for out (row j, col c)
#   A[4 + 130*(j+1+dy) + c + dx]
def _ab(t):
    dy, dx = t // 3 - 1, t % 3 - 1
    return 4 + PITCH * (1 + dy) + dx

# per-tensor tap split (tensor 0=q, 1=k, 2=v)
PE_TAPS = ((0, 8), (0, 8), (0, 2, 6, 8))
DVE_A_TAPS = ((1, 4, 7), (1, 4, 7), (1, 4, 7))      # even offsets on A
DVE_S_TAPS = ((2, 3, 5, 6), (2, 3, 5, 6), (3, 5))   # odd: even offsets on S
NPE = max(len(p) for p in PE_TAPS)

DWWIN = [(4 * w, 4) for w in range(HR // 4)]                         # 8 wins
CVWIN = [(4 * w, min(4, AR - 4 * w)) for w in range((AR + 3) // 4)]  # 9 wins
PJWIN = [(4 * w, 4) for w in range(HALF // 4)]                       # 16 wins


def _build():
    nc = bacc.Bacc(num_devices=8)

    x_ext = nc.declare_dram_parameter("x", [C, HALF + 2, W], BF16, isOutput=False)
    y_ext = nc.declare_dram_parameter("y", [C, HALF + 2, W], BF16, isOutput=False)
    wqT_ext = nc.declare_dram_parameter("wqT", [C, C], BF16, isOutput=False)
    wkT_ext = nc.declare_dram_parameter("wkT", [C, C], BF16, isOutput=False)
    wvT_ext = nc.declare_dram_parameter("wvT", [C, C], BF16, isOutput=False)
    wpT_ext = nc.declare_dram_parameter("wpT", [4, 96, C], BF16, isOutput=False)
    dwq_ext = nc.declare_dram_parameter("dwq", [C, 9], F32, isOutput=False)
    dwk_ext = nc.declare_dram_parameter("dwk", [C, 9], F32, isOutput=False)
    dwv_ext = nc.declare_dram_parameter("dwv", [C, 9], F32, isOutput=False)
    dg_ext = nc.declare_dram_parameter("dwdiag", [3, NPE, 3, 128, 128],
                                       BF16, isOutput=False)
    mask_ext = nc.declare_dram_parameter("blkmask", [96, 4 * 96], BF16,
                                         isOutput=False)
    tT_ext = nc.declare_dram_parameter("tempT", [1, C], F32, isOutput=False)
    out_ext = nc.declare_dram_parameter("out", [C, HALF * W], BF16, isOutput=True)

    SLEN = 96 * 4 * 96
    CCN = SLEN + 2 * C
    cc_in = nc.dram_tensor("cc_in", [1, CCN], F32)
    cc_out = nc.dram_tensor("cc_out", [2, CCN], F32)
    rn_scr = nc.dram_tensor("rn_scr", [2, C], F32)

    with TileContext(nc) as tc:
        with tc.tile_pool(name="const", bufs=1) as cpool:
            w_sb = []   # [tensor][kt] -> [128, C]
            for ti, ext in enumerate((wqT_ext, wkT_ext, wvT_ext)):
                tiles = []
                for kt in range(3):
                    t_ = cpool.tile([128, C], BF16, tag=f"w{ti}{kt}", name=f"w{ti}{kt}")
                    nc.scalar.dma_start(out=t_[:], in_=ext[128 * kt:128 * kt + 128, :])
                    tiles.append(t_)
                w_sb.append(tiles)
            wp_sb = [cpool.tile([96, C], BF16, tag=f"wp{g}", name=f"wp{g}")
                     for g in range(4)]
            for g in range(4):
                nc.scalar.dma_start(out=wp_sb[g][:], in_=wpT_ext[g])
            dw_sb = []  # [tensor][pt] -> [128, 9] f32
            for ti, ext in enumerate((dwq_ext, dwk_ext, dwv_ext)):
                tiles = []
                for pt in range(3):
                    t_ = cpool.tile([128, 9], F32, tag=f"dw{ti}{pt}", name=f"dw{ti}{pt}")
                    nc.scalar.dma_start(out=t_[:],
                                        in_=ext[128 * pt:128 * pt + 128, :])
                    tiles.append(t_)
                dw_sb.append(tiles)
            dg_sb = [[[cpool.tile([128, 128], BF16, tag=f"dg{ti}{i}{pt}",
                                  name=f"dg{ti}{i}{pt}")
                       for pt in range(3)]
                      for i in range(len(PE_TAPS[ti]))] for ti in range(3)]
            for ti in range(3):
                for i in range(len(PE_TAPS[ti])):
                    for pt in range(3):
                        nc.scalar.dma_start(out=dg_sb[ti][i][pt][:],
                                            in_=dg_ext[ti, i, pt])
            mask_sb = cpool.tile([96, 4 * 96], BF16, tag="mask", name="mask")
            nc.scalar.dma_start(out=mask_sb[:], in_=mask_ext[:])
            tT_sb = cpool.tile([1, C], F32, tag="tempT", name="tempT")
            nc.scalar.dma_start(out=tT_sb[:], in_=tT_ext[:])

            nsq_q = cpool.tile([128, 6], F32, tag="nsqq", name="nsqq")
            nsq_k = cpool.tile([128, 6], F32, tag="nsqk", name="nsqk")
            sp_sb = cpool.tile([96, 4 * 96], F32, tag="spsb", name="spsb")
            mT_sb = [cpool.tile([128, C], BF16, tag=f"mT{kt}", name=f"mT{kt}")
                     for kt in range(3)]

            # ---------------- per-half-tensor worker ----------------
            def half_pass(pool, psp, ti, src_ext, h, accs, acc_off, nsq):
                """conv1x1 + depthwise for rows [32h, 32h+32).
                accs[pt]: tile AP target for dw output (pitch-128);
                acc_off: element offset of this half inside accs[pt]."""
                src = []
                for kt in range(3):
                    s_ = pool.tile([128, AR * W], BF16, tag=f"src{kt}",
                                   name=f"src{kt}", bufs=1)
                    nc.sync.dma_start(
                        out=s_[:], in_=src_ext[128 * kt:128 * kt + 128,
                                               HR * h:HR * h + AR, :])
                    src.append(s_)
                As, Ss = [], []
                for pt in range(3):
                    A = pool.tile([128, ASZ], BF16, tag=f"A{pt}", name=f"A{pt}")
                    A3 = A[:, 2:2 + AR * PITCH].rearrange(
                        "p (r c) -> p r c", c=PITCH)
                    nc.vector.memset(A3[:, :, 0:2], 0.0)
                    nc.vector.memset(A[:, 0:2], 0.0)
                    nc.vector.memset(A[:, 2 + AR * PITCH:], 0.0)
                    for w0, rw in CVWIN:
                        cps = psp.tile([128, 512], F32, tag="convps", name="convps")
                        for kt in range(3):
                            nc.tensor.matmul(
                                cps[:, 0:rw * W],
                                w_sb[ti][kt][:, 128 * pt:128 * pt + 128],
                                src[kt][:, w0 * W:(w0 + rw) * W],
                                start=(kt == 0), stop=(kt == 2))
                        nc.scalar.copy(
                            A3[:, w0:w0 + rw, 2:2 + W],
                            cps[:, 0:rw * W].rearrange("p (r c) -> p r c", c=W))
                    S = pool.tile([128, ASZ], BF16, tag=f"S{pt}", name=f"S{pt}",
                                  bufs=1)
                    nc.scalar.dma_start(out=S[:, 0:ASZ - 1], in_=A[:, 1:ASZ])
                    As.append(A)
                    Ss.append(S)
                for pt in range(3):
                    A, S = As[pt], Ss[pt]
                    acc = accs[pt]
                    for j0, rw in DWWIN:
                        dps = psp.tile([128, 512], F32, tag="dwps", name="dwps")
                        pe = PE_TAPS[ti]
                        for i, t in enumerate(pe):
                            b = _ab(t) + PITCH * j0
                            nc.tensor.matmul(
                                dps[:, 0:rw * W],
                                dg_sb[ti][i][pt][:],
                                A[:, b:b + rw * PITCH].rearrange(
                                    "p (r c) -> p r c", c=PITCH)[:, :, 0:W],
                                start=(i == 0), stop=(i == len(pe) - 1))
                        nc.scalar.copy(
                            acc[:, acc_off + j0 * W:acc_off + (j0 + rw) * W],
                            dps[:, 0:rw * W])
                    av = acc[:, acc_off:acc_off + ACC].rearrange(
                        "p (r c) -> p r c", c=W)
                    for t in DVE_A_TAPS[ti]:
                        nc.vector.scalar_tensor_tensor(
                            out=av,
                            in0=A[:, _ab(t):_ab(t) + HR * PITCH].rearrange(
                                "p (r c) -> p r c", c=PITCH)[:, :, 0:W],
                            scalar=dw_sb[ti][pt][:, t:t + 1],
                            in1=av, op0=ALU.mult, op1=ALU.add)
                    for t in DVE_S_TAPS[ti]:
                        b = _ab(t) - 1
                        nc.vector.scalar_tensor_tensor(
                            out=av,
                            in0=S[:, b:b + HR * PITCH].rearrange(
                                "p (r c) -> p r c", c=PITCH)[:, :, 0:W],
                            scalar=dw_sb[ti][pt][:, t:t + 1],
                            in1=av, op0=ALU.mult, op1=ALU.add)
                    if nsq is not None:
                        # S is dead after the taps; reuse it as Square scratch
                        nc.scalar.activation(
                            S[:, 0:ACC], acc[:, acc_off:acc_off + ACC],
                            AF.Square,
                            accum_out=nsq[:, 2 * pt + h:2 * pt + h + 1])

            # ================== q/k phase with Gram ==================
            with tc.tile_pool(name="qk", bufs=2) as pool, \
                 tc.tile_pool(name="ps1", bufs=2, space="PSUM") as psp, \
                 tc.tile_pool(name="gramp", bufs=1, space="PSUM") as gramp:
                s_ps = gramp.tile([96, 4, 96], F32, tag="sps", name="sps")
                for h in range(2):
                    acck = [pool.tile([128, ACC], BF16, tag=f"acc{pt}",
                                      name=f"kacc{pt}") for pt in range(3)]
                    half_pass(pool, psp, 1, y_ext, h, acck, 0, nsq_k)
                    kT = pool.tile([128, HR, C], BF16, tag="kT", name="kT",
                                   bufs=1)
                    for g in range(4):
                        for pt in range(3):
                            nc.sync.dma_start_transpose(
                                kT[:, 8 * g:8 * g + 8, 128 * pt:128 * pt + 128],
                                acck[pt][:, 1024 * g:1024 * g + 1024])
                    accq = [pool.tile([128, ACC], BF16, tag=f"acc{pt}",
                                      name=f"qacc{pt}") for pt in range(3)]
                    half_pass(pool, psp, 0, x_ext, h, accq, 0, nsq_q)
                    for g in range(8):
                        qT = pool.tile([128, 4, C], BF16, tag="qT", name="qT")
                        for pt in range(3):
                            nc.scalar.dma_start_transpose(
                                qT[:, :, 128 * pt:128 * pt + 128],
                                accq[pt][:, 512 * g:512 * g + 512])
                        for j in range(4):
                            r = HR * h + 4 * g + j
                            for p in range(4):
                                nc.tensor.matmul(
                                    s_ps[:, p, :],
                                    qT[:, j, 96 * p:96 * p + 96],
                                    kT[:, 4 * g + j, 96 * p:96 * p + 96],
                                    start=(r == 0), stop=(r == HALF - 1),
                                    skip_group_check=True)
                nc.scalar.copy(
                    sp_sb[:].rearrange("p (g n) -> p g n", n=96), s_ps[:])

            # ============ v phase + collective + softmax + proj ============
            nsqr_q = cpool.tile([128, 3], F32, tag="nsqrq", name="nsqrq")
            nsqr_k = cpool.tile([128, 3], F32, tag="nsqrk", name="nsqrk")
            nc.vector.tensor_tensor(
                out=nsqr_q[:],
                in0=nsq_q[:].rearrange("p (t h) -> p t h", h=2)[:, :, 0],
                in1=nsq_q[:].rearrange("p (t h) -> p t h", h=2)[:, :, 1],
                op=ALU.add)
            nc.vector.tensor_tensor(
                out=nsqr_k[:],
                in0=nsq_k[:].rearrange("p (t h) -> p t h", h=2)[:, :, 0],
                in1=nsq_k[:].rearrange("p (t h) -> p t h", h=2)[:, :, 1],
                op=ALU.add)

            ccs = [cpool.tile([96, 4 * 96], F32, tag=f"ccs{r}", name=f"ccs{r}")
                   for r in range(2)]
            ccnq = [cpool.tile([128, 3], F32, tag=f"ccnq{r}", name=f"ccnq{r}")
                    for r in range(2)]
            ccnk = [cpool.tile([128, 3], F32, tag=f"ccnk{r}", name=f"ccnk{r}")
                    for r in range(2)]

            with tc.tile_pool(name="vp", bufs=2) as pool, \
                 tc.tile_pool(name="ps2", bufs=2, space="PSUM") as psp, \
                 tc.tile_pool(name="vstp", bufs=1) as vstp, \
                 tc.tile_pool(name="smp", bufs=1) as smp:
                vst = [vstp.tile([128, HALF * W], BF16, tag=f"vst{pt}",
                                 name=f"vst{pt}") for pt in range(3)]
                for h in range(2):
                    half_pass(pool, psp, 2, y_ext, h, vst, ACC * h, None)

                # ---- collective (gpsimd-only critical section) ----
                with tc.tile_critical():
                    ccsem = nc.alloc_semaphore("ccsem")
                    sv = 0
                    nc.gpsimd.dma_start(
                        out=cc_in[0, 0:SLEN].rearrange("(p n) -> p n", p=96),
                        in_=sp_sb[:]).then_inc(ccsem, 16)
                    sv += 16
                    nc.gpsimd.dma_start(
                        out=cc_in[0, SLEN:SLEN + C].rearrange(
                            "(n p) -> p n", p=128),
                        in_=nsqr_q[:]).then_inc(ccsem, 16)
                    sv += 16
                    nc.gpsimd.dma_start(
                        out=cc_in[0, SLEN + C:].rearrange(
                            "(n p) -> p n", p=128),
                        in_=nsqr_k[:]).then_inc(ccsem, 16)
                    sv += 16
                    nc.gpsimd.wait_ge(ccsem, sv)
                    nc.gpsimd.collective_compute(
                        "AllGather", ALU.bypass,
                        replica_groups=[[0, 1], [2, 3], [4, 5], [6, 7]],
                        ins=[cc_in[:].opt()],
                        outs=[cc_out[:].opt()],
                    ).then_inc(ccsem, 1)
                    sv += 1
                    nc.gpsimd.wait_ge(ccsem, sv)
                    for r in range(2):
                        nc.gpsimd.dma_start(
                            out=ccs[r][:],
                            in_=cc_out[r, 0:SLEN].rearrange(
                                "(p n) -> p n", p=96)).then_inc(ccsem, 16)
                        sv += 16
                        nc.gpsimd.dma_start(
                            out=ccnq[r][:],
                            in_=cc_out[r, SLEN:SLEN + C].rearrange(
                                "(n p) -> p n", p=128)).then_inc(ccsem, 16)
                        sv += 16
                        nc.gpsimd.dma_start(
                            out=ccnk[r][:],
                            in_=cc_out[r, SLEN + C:].rearrange(
                                "(n p) -> p n", p=128)).then_inc(ccsem, 16)
                        sv += 16
                    nc.gpsimd.wait_ge(ccsem, sv)

                # -------------------- softmax --------------------
                s_full = smp.tile([96, 4, 96], F32, tag="sfull", name="sfull")
                nc.vector.tensor_tensor(
                    out=s_full[:],
                    in0=ccs[0][:].rearrange("p (g n) -> p g n", n=96),
                    in1=ccs[1][:].rearrange("p (g n) -> p g n", n=96),
                    op=ALU.add)
                rnq = smp.tile([128, 3], F32, tag="rnq", name="rnq")
                rnk = smp.tile([128, 3], F32, tag="rnk", name="rnk")
                nc.vector.tensor_tensor(out=rnq[:], in0=ccnq[0][:],
                                        in1=ccnq[1][:], op=ALU.add)
                nc.vector.tensor_tensor(out=rnk[:], in0=ccnk[0][:],
                                        in1=ccnk[1][:], op=ALU.add)
                nc.scalar.activation(rnq[:], rnq[:], AF.Sqrt)
                nc.scalar.activation(rnk[:], rnk[:], AF.Sqrt)
                nc.vector.tensor_scalar_max(rnq[:], rnq[:], 1e-12)
                nc.vector.tensor_scalar_max(rnk[:], rnk[:], 1e-12)
                nc.vector.reciprocal(rnq[:], rnq[:])
                nc.vector.reciprocal(rnk[:], rnk[:])

                rnqT = smp.tile([1, C], F32, tag="rnqT", name="rnqT")
                rnkT = smp.tile([1, C], F32, tag="rnkT", name="rnkT")
                with tc.tile_critical():
                    rsem = nc.alloc_semaphore("rsem")
                    nc.gpsimd.dma_start(
                        out=rn_scr[0, :].rearrange("(n p) -> p n", p=128),
                        in_=rnq[:]).then_inc(rsem, 16)
                    nc.gpsimd.dma_start(
                        out=rn_scr[1, :].rearrange("(n p) -> p n", p=128),
                        in_=rnk[:]).then_inc(rsem, 16)
                    nc.gpsimd.wait_ge(rsem, 32)
                    nc.gpsimd.dma_start(
                        out=rnqT[:], in_=rn_scr[0:1, :]).then_inc(rsem, 16)
                    nc.gpsimd.dma_start(
                        out=rnkT[:], in_=rn_scr[1:2, :]).then_inc(rsem, 16)
                    nc.gpsimd.wait_ge(rsem, 64)
                nc.vector.tensor_tensor(out=rnqT[:], in0=rnqT[:],
                                        in1=tT_sb[:], op=ALU.mult)

                outer_ps = psp.tile([96, 4, 96], F32, tag="outerps",
                                    name="outerps", bufs=1)
                for p in range(4):
                    nc.tensor.matmul(
                        outer_ps[:, p, :],
                        rnqT[0:1, 96 * p:96 * p + 96],
                        rnkT[0:1, 96 * p:96 * p + 96],
                        start=True, stop=True)
                logits = smp.tile([96, 4, 96], F32, tag="logits", name="logits")
                nc.vector.tensor_tensor(out=logits[:], in0=s_full[:],
                                        in1=outer_ps[:], op=ALU.mult)
                expv = smp.tile([96, 4 * 96], F32, tag="expv", name="expv")
                nc.scalar.activation(
                    expv[:], logits[:].rearrange("p g n -> p (g n)"), AF.Exp)
                expm = smp.tile([96, 4, 96], F32, tag="expm", name="expm")
                nc.vector.tensor_tensor(
                    out=expm[:],
                    in0=expv[:].rearrange("p (g n) -> p g n", n=96),
                    in1=mask_sb[:].rearrange("p (g n) -> p g n", n=96),
                    op=ALU.mult)
                rs = smp.tile([96, 4], F32, tag="rs", name="rs")
                nc.vector.tensor_reduce(out=rs[:], in_=expm[:],
                                        axis=mybir.AxisListType.X, op=ALU.add)
                nc.vector.reciprocal(rs[:], rs[:])
                attn = smp.tile([96, 4, 96], BF16, tag="attn", name="attn")
                for p in range(4):
                    nc.vector.tensor_scalar(
                        attn[:, p, :], expm[:, p, :], rs[:, p:p + 1],
                        None, ALU.mult)

                # ---- fold attention into Wproj: mT[cd, o] ----
                for g in range(4):
                    # attn's off-diagonal 48x48 blocks are zero (masked), so
                    # one 96x96 matmul per pair-block is exact.
                    mps = psp.tile([96, C], F32, tag="mps", name="mps", bufs=1)
                    nc.tensor.matmul(
                        mps[:], attn[:, g, :], wp_sb[g][:],
                        start=True, stop=True)
                    for q in range(3):  # 32-part pieces (PSUM window rule)
                        r0 = 96 * g + 32 * q
                        nc.scalar.copy(
                            mT_sb[r0 // 128][r0 % 128:r0 % 128 + 32, :],
                            mps[32 * q:32 * q + 32, :])

                # ------------- proj: out = mT^T @ vdw -------------
                for ot in range(3):
                    for j0, rw in PJWIN:
                        pps = psp.tile([128, 512], F32, tag="projps",
                                       name="pps")
                        for kt in range(3):
                            nc.tensor.matmul(
                                pps[:, 0:rw * W],
                                mT_sb[kt][:, 128 * ot:128 * ot + 128],
                                vst[kt][:, j0 * W:(j0 + rw) * W],
                                start=(kt == 0), stop=(kt == 2))
                        osb = pool.tile([128, 512], BF16, tag="osb", name="osb")
                        nc.scalar.copy(osb[:, 0:rw * W], pps[:, 0:rw * W])
                        nc.sync.dma_start(
                            out=out_ext[128 * ot:128 * ot + 128,
                                        j0 * W:(j0 + rw) * W],
                            in_=osb[:, 0:rw * W])
    return nc


_BUILD_CACHE = {}


def _get_program():
    if "nc" not in _BUILD_CACHE:
        nc = _build()
        if not nc.is_finalized():
            nc.finalize()
        _BUILD_CACHE["nc"] = nc
    return _BUILD_CACHE["nc"]


def kernel(x, y, Wq, Wkv, Wdw, Wproj, temperature):
    B, C_, H, W_ = x.shape
    assert C_ == C and W_ == W and H == 2 * HALF
    nc = _get_program()

    f32 = np.float32
    bf16 = ml_dtypes.bfloat16
    x = np.asarray(x, f32)
    y = np.asarray(y, f32)
    Wq = np.asarray(Wq, f32)
    Wkv = np.asarray(Wkv, f32)
    Wdw = np.asarray(Wdw, f32)
    Wproj = np.asarray(Wproj, f32)
    temperature = np.asarray(temperature, f32)

    wqT = np.ascontiguousarray(Wq.T).astype(bf16)
    wkT = np.ascontiguousarray(Wkv[:C].T).astype(bf16)
    wvT = np.ascontiguousarray(Wkv[C:].T).astype(bf16)
    wpT = np.ascontiguousarray(Wproj.T.reshape(4, 96, C)).astype(bf16)
    dwq = np.ascontiguousarray(Wdw[0:C, 0].reshape(C, 9))
    dwk = np.ascontiguousarray(Wdw[C:2 * C, 0].reshape(C, 9))
    dwv = np.ascontiguousarray(Wdw[2 * C:, 0].reshape(C, 9))

    dwdiag = np.zeros((3, NPE, 3, 128, 128), f32)
    for ti, dwt in enumerate((dwq, dwk, dwv)):
        for i, t in enumerate(PE_TAPS[ti]):
            for pt in range(3):
                dwdiag[ti, i, pt][np.arange(128), np.arange(128)] = \
                    dwt[128 * pt:128 * pt + 128, t]
    dwdiag = dwdiag.astype(bf16)

    blk = np.zeros((96, 4 * 96), f32)
    for p in range(4):
        blk[0:48, 96 * p:96 * p + 48] = 1.0
        blk[48:96, 96 * p + 48:96 * p + 96] = 1.0
    blkmask = blk.astype(bf16)
    tempT = np.repeat(temperature.reshape(HEADS), C // HEADS).reshape(1, C)
    tempT = tempT.astype(f32)

    in_maps = []
    for c in range(8):
        b, half = c // 2, c % 2
        r0 = half * HALF

        def shard(t):
            s = np.zeros((C, HALF + 2, W_), f32)
            s[:, 1:HALF + 1] = t[b, :, r0:r0 + HALF]
            if r0 > 0:
                s[:, 0] = t[b, :, r0 - 1]
            if r0 + HALF < H:
                s[:, HALF + 1] = t[b, :, r0 + HALF]
            return s.astype(bf16)

        in_maps.append({
            "x": shard(x), "y": shard(y),
            "wqT": wqT, "wkT": wkT, "wvT": wvT, "wpT": wpT,
            "dwq": dwq, "dwk": dwk, "dwv": dwv,
            "dwdiag": dwdiag, "blkmask": blkmask,
            "tempT": tempT,
        })

    import os
    kw = {}
    if os.environ.get("KBENCH_TRACE"):
        kw = dict(trace=True)
    res = run_bass_kernel_spmd(nc, in_maps, list(range(8)), **kw)
    kernel._last_result = res

    out = np.zeros((B, C, H, W_), f32)
    for c in range(8):
        b, half = c // 2, c % 2
        out[b, :, half * HALF:(half + 1) * HALF] = \
            np.asarray(res.results[c]["out"], f32).reshape(C, HALF, W_)
    return out


# revision 18
# speedup vs baseline: 1.0319x; 1.0319x over previous
"""Trainium2 Bass kernel for nn_CrossAttention (dense_transformer).  v3

Sharding: 8 cores = (batch 0..3) x (image half 0..1), 64 rows + 1 halo row
each side per core.  Only per-head Gram matrices and l2-norm square sums
are AllGather'd between the two cores of a batch.

v3 structure (vs v2, from trace analysis: DVE 69%/PE 65%/ACT 56% busy):
  - conv1x1 outputs land in a 130-pitch padded SBUF layout A (2 zero pad
    cols between rows, row base 4B-aligned), conv psum f32 N=512.
  - depthwise 9 taps split: 2 taps (q,k) / 4 taps (v) on PE as chained
    diagonal matmuls into bf16 psum (alignment-free); the ACT psum->acc
    copy doubles as the accumulator init; remaining taps are single-pass
    DVE scalar_tensor_tensor (2x mode): dx==0 taps read A at even offsets,
    odd taps read S = A<<1 (built by DMA, SBUF->SBUF) at even offsets.
  - acc (dw output) is padless pitch-128: flat DVE out APs, flat norms,
    flat proj windows, and batched DMA-xbar transposes ([128, 1024] ->
    [128, 8, 128] per-block) straight from acc -- no PE transposes.
  - Gram accumulated per spatial row over transposed q/k tiles (64 rows
    x 4 pair-blocks); q,k processed in interleaved 32-row half-tensors.
  - attention is folded into Wproj: M^T = A_head^T-weighted WprojT built
    on-PE after softmax; final conv1x1 uses M^T directly on the stored
    depthwise-v (attn@v matmuls and their copies disappear).
"""
import numpy as np
import ml_dtypes

import concourse.bass as bass
import concourse.bacc as bacc
from concourse import mybir
from concourse.bass_utils import run_bass_kernel_spmd
from concourse.tile import TileContext

F32 = mybir.dt.float32
BF16 = mybir.dt.bfloat16
ALU = mybir.AluOpType
AF = mybir.ActivationFunctionType

C = 384
HEADS = 8
W = 128
HALF = 64
HR = 32                  # output rows per half-tensor pass
AR = HR + 2              # conv rows per half (1 halo each side)
PITCH = 130              # A row pitch: [pad, pad, 128 data]
ASZ = 2 + AR * PITCH + 6   # 4428; leading/trailing zero guards
ACC = HR * W             # 4096, padless acc per half

# tap t: dy = t//3-1, dx = t%3-1; base offset into A for out (row j, col c)
#   A[4 + 130*(j+1+dy) + c + dx]
def _ab(t):
    dy, dx = t // 3 - 1, t % 3 - 1
    return 4 + PITCH * (1 + dy) + dx

# per-tensor tap split (tensor 0=q, 1=k, 2=v)
PE_TAPS = ((0, 8), (0, 8), (0, 2, 6, 8))
DVE_A_TAPS = ((1, 4, 7), (1, 4, 7), (1, 4, 7))      # even offsets on A
DVE_S_TAPS = ((2, 3, 5, 6), (2, 3, 5, 6), (3, 5))   # odd: even offsets on S
NPE = max(len(p) for p in PE_TAPS)

DWWIN = [(4 * w, 4) for w in range(HR // 4)]                         # 8 wins
CVWIN = [(4 * w, min(4, AR - 4 * w)) for w in range((AR + 3) // 4)]  # 9 wins
PJWIN = [(4 * w, 4) for w in range(HALF // 4)]                       # 16 wins


def _build():
    nc = bacc.Bacc(num_devices=8)

    x_ext = nc.declare_dram_parameter("x", [C, HALF + 2, W], BF16, isOutput=False)
    y_ext = nc.declare_dram_parameter("y", [C, HALF + 2, W], BF16, isOutput=False)
    wqT_ext = nc.declare_dram_parameter("wqT", [C, C], BF16, isOutput=False)
    wkT_ext = nc.declare_dram_parameter("wkT", [C, C], BF16, isOutput=False)
    wvT_ext = nc.declare_dram_parameter("wvT", [C, C], BF16, isOutput=False)
    wpT_ext = nc.declare_dram_parameter("wpT", [4, 96, C], BF16, isOutput=False)
    dwq_ext = nc.declare_dram_parameter("dwq", [C, 9], BF16, isOutput=False)
    dwk_ext = nc.declare_dram_parameter("dwk", [C, 9], BF16, isOutput=False)
    dwv_ext = nc.declare_dram_parameter("dwv", [C, 9], BF16, isOutput=False)
    dg_ext = nc.declare_dram_parameter("dwdiag", [3, NPE, 3, 128, 128],
                                       BF16, isOutput=False)
    mask_ext = nc.declare_dram_parameter("blkmask", [96, 4 * 96], BF16,
                                         isOutput=False)
    tT_ext = nc.declare_dram_parameter("tempT", [1, C], F32, isOutput=False)
    out_ext = nc.declare_dram_parameter("out", [C, HALF * W], BF16, isOutput=True)

    SLEN = 96 * 4 * 96
    CCN = SLEN + 2 * C
    cc_in = nc.dram_tensor("cc_in", [1, CCN], F32)
    cc_out = nc.dram_tensor("cc_out", [2, CCN], F32)
    rn_scr = nc.dram_tensor("rn_scr", [2, C], F32)

    with TileContext(nc) as tc:
        with tc.tile_pool(name="const", bufs=1) as cpool:
            w_sb = []   # [tensor][kt] -> [128, C]
            for ti, ext in enumerate((wqT_ext, wkT_ext, wvT_ext)):
                tiles = []
                for kt in range(3):
                    t_ = cpool.tile([128, C], BF16, tag=f"w{ti}{kt}", name=f"w{ti}{kt}")
                    nc.scalar.dma_start(out=t_[:], in_=ext[128 * kt:128 * kt + 128, :])
                    tiles.append(t_)
                w_sb.append(tiles)
            wp_sb = [cpool.tile([96, C], BF16, tag=f"wp{g}", name=f"wp{g}")
                     for g in range(4)]
            for g in range(4):
                nc.scalar.dma_start(out=wp_sb[g][:], in_=wpT_ext[g])
            dw_sb = []  # [tensor][pt] -> [128, 9] f32
            for ti, ext in enumerate((dwq_ext, dwk_ext, dwv_ext)):
                tiles = []
                for pt in range(3):
                    t_ = cpool.tile([128, 9], BF16, tag=f"dw{ti}{pt}", name=f"dw{ti}{pt}")
                    nc.scalar.dma_start(out=t_[:],
                                        in_=ext[128 * pt:128 * pt + 128, :])
                    tiles.append(t_)
                dw_sb.append(tiles)
            dg_sb = [[[cpool.tile([128, 128], BF16, tag=f"dg{ti}{i}{pt}",
                                  name=f"dg{ti}{i}{pt}")
                       for pt in range(3)]
                      for i in range(len(PE_TAPS[ti]))] for ti in range(3)]
            for ti in range(3):
                for i in range(len(PE_TAPS[ti])):
                    for pt in range(3):
                        nc.scalar.dma_start(out=dg_sb[ti][i][pt][:],
                                            in_=dg_ext[ti, i, pt])
            mask_sb = cpool.tile([96, 4 * 96], BF16, tag="mask", name="mask")
            nc.scalar.dma_start(out=mask_sb[:], in_=mask_ext[:])
            tT_sb = cpool.tile([1, C], F32, tag="tempT", name="tempT")
            nc.scalar.dma_start(out=tT_sb[:], in_=tT_ext[:])

            nsq_q = cpool.tile([128, 6], F32, tag="nsqq", name="nsqq")
            nsq_k = cpool.tile([128, 6], F32, tag="nsqk", name="nsqk")
            sp_sb = cpool.tile([96, 4 * 96], F32, tag="spsb", name="spsb")
            mT_sb = [cpool.tile([128, C], BF16, tag=f"mT{kt}", name=f"mT{kt}")
                     for kt in range(3)]

            # ---------------- per-half-tensor worker ----------------
            def half_pass(pool, psp, ti, src_ext, h, accs, acc_off, nsq):
                """conv1x1 + depthwise for rows [32h, 32h+32).
                accs[pt]: tile AP target for dw output (pitch-128);
                acc_off: element offset of this half inside accs[pt]."""
                src = []
                for kt in range(3):
                    s_ = pool.tile([128, AR * W], BF16, tag=f"src{kt}",
                                   name=f"src{kt}", bufs=1)
                    nc.sync.dma_start(
                        out=s_[:], in_=src_ext[128 * kt:128 * kt + 128,
                                               HR * h:HR * h + AR, :])
                    src.append(s_)
                As, Ss = [], []
                for pt in range(3):
                    A = pool.tile([128, ASZ], BF16, tag=f"A{pt}", name=f"A{pt}")
                    A3 = A[:, 2:2 + AR * PITCH].rearrange(
                        "p (r c) -> p r c", c=PITCH)
                    nc.vector.memset(A3[:, :, 0:2], 0.0)
                    nc.vector.memset(A[:, 0:2], 0.0)
                    nc.vector.memset(A[:, 2 + AR * PITCH:], 0.0)
                    for w0, rw in CVWIN:
                        cps = psp.tile([128, 512], F32, tag="convps", name="convps")
                        for kt in range(3):
                            nc.tensor.matmul(
                                cps[:, 0:rw * W],
                                w_sb[ti][kt][:, 128 * pt:128 * pt + 128],
                                src[kt][:, w0 * W:(w0 + rw) * W],
                                start=(kt == 0), stop=(kt == 2))
                        nc.scalar.copy(
                            A3[:, w0:w0 + rw, 2:2 + W],
                            cps[:, 0:rw * W].rearrange("p (r c) -> p r c", c=W))
                    S = pool.tile([128, ASZ], BF16, tag=f"S{pt}", name=f"S{pt}",
                                  bufs=1)
                    nc.scalar.dma_start(out=S[:, 0:ASZ - 1], in_=A[:, 1:ASZ])
                    As.append(A)
                    Ss.append(S)
                for pt in range(3):
                    A, S = As[pt], Ss[pt]
                    acc = accs[pt]
                    for j0, rw in DWWIN:
                        dps = psp.tile([128, 512], F32, tag="dwps", name="dwps")
                        pe = PE_TAPS[ti]
                        for i, t in enumerate(pe):
                            b = _ab(t) + PITCH * j0
                            nc.tensor.matmul(
                                dps[:, 0:rw * W],
                                dg_sb[ti][i][pt][:],
                                A[:, b:b + rw * PITCH].rearrange(
                                    "p (r c) -> p r c", c=PITCH)[:, :, 0:W],
                                start=(i == 0), stop=(i == len(pe) - 1))
                        nc.scalar.copy(
                            acc[:, acc_off + j0 * W:acc_off + (j0 + rw) * W],
                            dps[:, 0:rw * W])
                    av = acc[:, acc_off:acc_off + ACC].rearrange(
                        "p (r c) -> p r c", c=W)
                    for t in DVE_A_TAPS[ti]:
                        nc.vector.scalar_tensor_tensor(
                            out=av,
                            in0=A[:, _ab(t):_ab(t) + HR * PITCH].rearrange(
                                "p (r c) -> p r c", c=PITCH)[:, :, 0:W],
                            scalar=dw_sb[ti][pt][:, t:t + 1],
                            in1=av, op0=ALU.mult, op1=ALU.add)
                    for t in DVE_S_TAPS[ti]:
                        b = _ab(t) - 1
                        nc.vector.scalar_tensor_tensor(
                            out=av,
                            in0=S[:, b:b + HR * PITCH].rearrange(
                                "p (r c) -> p r c", c=PITCH)[:, :, 0:W],
                            scalar=dw_sb[ti][pt][:, t:t + 1],
                            in1=av, op0=ALU.mult, op1=ALU.add)
                    if nsq is not None:
                        # S is dead after the taps; reuse it as Square scratch
                        nc.scalar.activation(
                            S[:, 0:ACC], acc[:, acc_off:acc_off + ACC],
                            AF.Square,
                            accum_out=nsq[:, 2 * pt + h:2 * pt + h + 1])

            # ================== q/k phase with Gram ==================
            with tc.tile_pool(name="qk", bufs=2) as pool, \
                 tc.tile_pool(name="ps1", bufs=2, space="PSUM") as psp, \
                 tc.tile_pool(name="gramp", bufs=1, space="PSUM") as gramp:
                s_ps = gramp.tile([96, 4, 96], F32, tag="sps", name="sps")
                for h in range(2):
                    acck = [pool.tile([128, ACC], BF16, tag=f"acc{pt}",
                                      name=f"kacc{pt}") for pt in range(3)]
                    half_pass(pool, psp, 1, y_ext, h, acck, 0, nsq_k)
                    kT = pool.tile([128, HR, C], BF16, tag="kT", name="kT",
                                   bufs=1)
                    for pt in range(3):
                        nc.sync.dma_start_transpose(
                            kT[:, :, 128 * pt:128 * pt + 128],
                            acck[pt][:, 0:ACC])
                    accq = [pool.tile([128, ACC], BF16, tag=f"acc{pt}",
                                      name=f"qacc{pt}") for pt in range(3)]
                    half_pass(pool, psp, 0, x_ext, h, accq, 0, nsq_q)
                    for g in range(4):
                        qT = pool.tile([128, 8, C], BF16, tag="qT", name="qT",
                                       bufs=1)
                        for pt in range(3):
                            nc.sync.dma_start_transpose(
                                qT[:, :, 128 * pt:128 * pt + 128],
                                accq[pt][:, 1024 * g:1024 * g + 1024])
                        for j in range(8):
                            r = HR * h + 8 * g + j
                            for p in range(4):
                                nc.tensor.matmul(
                                    s_ps[:, p, :],
                                    qT[:, j, 96 * p:96 * p + 96],
                                    kT[:, 8 * g + j, 96 * p:96 * p + 96],
                                    start=(r == 0), stop=(r == HALF - 1),
                                    skip_group_check=True)
                nc.scalar.copy(
                    sp_sb[:].rearrange("p (g n) -> p g n", n=96), s_ps[:])

            # ============ v phase + collective + softmax + proj ============
            nsqr_q = cpool.tile([128, 3], F32, tag="nsqrq", name="nsqrq")
            nsqr_k = cpool.tile([128, 3], F32, tag="nsqrk", name="nsqrk")
            nc.vector.tensor_tensor(
                out=nsqr_q[:],
                in0=nsq_q[:].rearrange("p (t h) -> p t h", h=2)[:, :, 0],
                in1=nsq_q[:].rearrange("p (t h) -> p t h", h=2)[:, :, 1],
                op=ALU.add)
            nc.vector.tensor_tensor(
                out=nsqr_k[:],
                in0=nsq_k[:].rearrange("p (t h) -> p t h", h=2)[:, :, 0],
                in1=nsq_k[:].rearrange("p (t h) -> p t h", h=2)[:, :, 1],
                op=ALU.add)

            ccs = [cpool.tile([96, 4 * 96], F32, tag=f"ccs{r}", name=f"ccs{r}")
                   for r in range(2)]
            ccnq = [cpool.tile([128, 3], F32, tag=f"ccnq{r}", name=f"ccnq{r}")
                    for r in range(2)]
            ccnk = [cpool.tile([128, 3], F32, tag=f"ccnk{r}", name=f"ccnk{r}")
                    for r in range(2)]

            with tc.tile_pool(name="vp", bufs=2) as pool, \
                 tc.tile_pool(name="ps2", bufs=2, space="PSUM") as psp, \
                 tc.tile_pool(name="vstp", bufs=1) as vstp, \
                 tc.tile_pool(name="smp", bufs=1) as smp:
                vst = [vstp.tile([128, HALF * W], BF16, tag=f"vst{pt}",
                                 name=f"vst{pt}") for pt in range(3)]
                for h in range(2):
                    half_pass(pool, psp, 2, y_ext, h, vst, ACC * h, None)

                # ---- collective (gpsimd-only critical section) ----
                with tc.tile_critical():
                    ccsem = nc.alloc_semaphore("ccsem")
                    sv = 0
                    nc.gpsimd.dma_start(
                        out=cc_in[0, 0:SLEN].rearrange("(p n) -> p n", p=96),
                        in_=sp_sb[:]).then_inc(ccsem, 16)
                    sv += 16
                    nc.gpsimd.dma_start(
                        out=cc_in[0, SLEN:SLEN + C].rearrange(
                            "(n p) -> p n", p=128),
                        in_=nsqr_q[:]).then_inc(ccsem, 16)
                    sv += 16
                    nc.gpsimd.dma_start(
                        out=cc_in[0, SLEN + C:].rearrange(
                            "(n p) -> p n", p=128),
                        in_=nsqr_k[:]).then_inc(ccsem, 16)
                    sv += 16
                    nc.gpsimd.wait_ge(ccsem, sv)
                    nc.gpsimd.collective_compute(
                        "AllGather", ALU.bypass,
                        replica_groups=[[0, 1], [2, 3], [4, 5], [6, 7]],
                        ins=[cc_in[:].opt()],
                        outs=[cc_out[:].opt()],
                    ).then_inc(ccsem, 1)
                    sv += 1
                    nc.gpsimd.wait_ge(ccsem, sv)
                    for r in range(2):
                        nc.gpsimd.dma_start(
                            out=ccs[r][:],
                            in_=cc_out[r, 0:SLEN].rearrange(
                                "(p n) -> p n", p=96)).then_inc(ccsem, 16)
                        sv += 16
                        nc.gpsimd.dma_start(
                            out=ccnq[r][:],
                            in_=cc_out[r, SLEN:SLEN + C].rearrange(
                                "(n p) -> p n", p=128)).then_inc(ccsem, 16)
                        sv += 16
                        nc.gpsimd.dma_start(
                            out=ccnk[r][:],
                            in_=cc_out[r, SLEN + C:].rearrange(
                                "(n p) -> p n", p=128)).then_inc(ccsem, 16)
                        sv += 16
                    nc.gpsimd.wait_ge(ccsem, sv)

                # -------------------- softmax --------------------
                s_full = smp.tile([96, 4, 96], F32, tag="sfull", name="sfull")
                nc.vector.tensor_tensor(
                    out=s_full[:],
                    in0=ccs[0][:].rearrange("p (g n) -> p g n", n=96),
                    in1=ccs[1][:].rearrange("p (g n) -> p g n", n=96),
                    op=ALU.add)
                rnq = smp.tile([128, 3], F32, tag="rnq", name="rnq")
                rnk = smp.tile([128, 3], F32, tag="rnk", name="rnk")
                nc.vector.tensor_tensor(out=rnq[:], in0=ccnq[0][:],
                                        in1=ccnq[1][:], op=ALU.add)
                nc.vector.tensor_tensor(out=rnk[:], in0=ccnk[0][:],
                                        in1=ccnk[1][:], op=ALU.add)
                nc.scalar.activation(rnq[:], rnq[:], AF.Sqrt)
                nc.scalar.activation(rnk[:], rnk[:], AF.Sqrt)
                nc.vector.tensor_scalar_max(rnq[:], rnq[:], 1e-12)
                nc.vector.tensor_scalar_max(rnk[:], rnk[:], 1e-12)
                nc.vector.reciprocal(rnq[:], rnq[:])
                nc.vector.reciprocal(rnk[:], rnk[:])

                rnqT = smp.tile([1, C], F32, tag="rnqT", name="rnqT")
                rnkT = smp.tile([1, C], F32, tag="rnkT", name="rnkT")
                with tc.tile_critical():
                    rsem = nc.alloc_semaphore("rsem")
                    nc.gpsimd.dma_start(
                        out=rn_scr[0, :].rearrange("(n p) -> p n", p=128),
                        in_=rnq[:]).then_inc(rsem, 16)
                    nc.gpsimd.dma_start(
                        out=rn_scr[1, :].rearrange("(n p) -> p n", p=128),
                        in_=rnk[:]).then_inc(rsem, 16)
                    nc.gpsimd.wait_ge(rsem, 32)
                    nc.gpsimd.dma_start(
                        out=rnqT[:], in_=rn_scr[0:1, :]).then_inc(rsem, 16)
                    nc.gpsimd.dma_start(
                        out=rnkT[:], in_=rn_scr[1:2, :]).then_inc(rsem, 16)
                    nc.gpsimd.wait_ge(rsem, 64)
                nc.vector.tensor_tensor(out=rnqT[:], in0=rnqT[:],
                                        in1=tT_sb[:], op=ALU.mult)

                outer_ps = psp.tile([96, 4, 96], F32, tag="outerps",
                                    name="outerps", bufs=1)
                for p in range(4):
                    nc.tensor.matmul(
                        outer_ps[:, p, :],
                        rnqT[0:1, 96 * p:96 * p + 96],
                        rnkT[0:1, 96 * p:96 * p + 96],
                        start=True, stop=True)
                logits = smp.tile([96, 4, 96], F32, tag="logits", name="logits")
                nc.vector.tensor_tensor(out=logits[:], in0=s_full[:],
                                        in1=outer_ps[:], op=ALU.mult)
                expv = smp.tile([96, 4 * 96], F32, tag="expv", name="expv")
                nc.scalar.activation(
                    expv[:], logits[:].rearrange("p g n -> p (g n)"), AF.Exp)
                expm = smp.tile([96, 4, 96], F32, tag="expm", name="expm")
                nc.vector.tensor_tensor(
                    out=expm[:],
                    in0=expv[:].rearrange("p (g n) -> p g n", n=96),
                    in1=mask_sb[:].rearrange("p (g n) -> p g n", n=96),
                    op=ALU.mult)
                rs = smp.tile([96, 4], F32, tag="rs", name="rs")
                nc.vector.tensor_reduce(out=rs[:], in_=expm[:],
                                        axis=mybir.AxisListType.X, op=ALU.add)
                nc.vector.reciprocal(rs[:], rs[:])
                attn = smp.tile([96, 4, 96], BF16, tag="attn", name="attn")
                for p in range(4):
                    nc.vector.tensor_scalar(
                        attn[:, p, :], expm[:, p, :], rs[:, p:p + 1],
                        None, ALU.mult)

                # ---- fold attention into Wproj: mT[cd, o] ----
                for g in range(4):
                    # attn's off-diagonal 48x48 blocks are zero (masked), so
                    # one 96x96 matmul per pair-block is exact.
                    mps = psp.tile([96, C], F32, tag="mps", name="mps", bufs=1)
                    nc.tensor.matmul(
                        mps[:], attn[:, g, :], wp_sb[g][:],
                        start=True, stop=True)
                    for q in range(3):  # 32-part pieces (PSUM window rule)
                        r0 = 96 * g + 32 * q
                        nc.scalar.copy(
                            mT_sb[r0 // 128][r0 % 128:r0 % 128 + 32, :],
                            mps[32 * q:32 * q + 32, :])

                # ------------- proj: out = mT^T @ vdw -------------
                for ot in range(3):
                    for j0, rw in PJWIN:
                        pps = psp.tile([128, 512], F32, tag="projps",
                                       name="pps")
                        for kt in range(3):
                            nc.tensor.matmul(
                                pps[:, 0:rw * W],
                                mT_sb[kt][:, 128 * ot:128 * ot + 128],
                                vst[kt][:, j0 * W:(j0 + rw) * W],
                                start=(kt == 0), stop=(kt == 2))
                        osb = pool.tile([128, 512], BF16, tag="osb", name="osb")
                        nc.scalar.copy(osb[:, 0:rw * W], pps[:, 0:rw * W])
                        nc.sync.dma_start(
                            out=out_ext[128 * ot:128 * ot + 128,
                                        j0 * W:(j0 + rw) * W],
                            in_=osb[:, 0:rw * W])
    return nc


_BUILD_CACHE = {}


def _get_program():
    if "nc" not in _BUILD_CACHE:
        nc = _build()
        if not nc.is_finalized():
            nc.finalize()
        _BUILD_CACHE["nc"] = nc
    return _BUILD_CACHE["nc"]


def kernel(x, y, Wq, Wkv, Wdw, Wproj, temperature):
    B, C_, H, W_ = x.shape
    assert C_ == C and W_ == W and H == 2 * HALF
    nc = _get_program()

    f32 = np.float32
    bf16 = ml_dtypes.bfloat16
    x = np.asarray(x, f32)
    y = np.asarray(y, f32)
    Wq = np.asarray(Wq, f32)
    Wkv = np.asarray(Wkv, f32)
    Wdw = np.asarray(Wdw, f32)
    Wproj = np.asarray(Wproj, f32)
    temperature = np.asarray(temperature, f32)

    wqT = np.ascontiguousarray(Wq.T).astype(bf16)
    wkT = np.ascontiguousarray(Wkv[:C].T).astype(bf16)
    wvT = np.ascontiguousarray(Wkv[C:].T).astype(bf16)
    wpT = np.ascontiguousarray(Wproj.T.reshape(4, 96, C)).astype(bf16)
    dwq = np.ascontiguousarray(Wdw[0:C, 0].reshape(C, 9))
    dwk = np.ascontiguousarray(Wdw[C:2 * C, 0].reshape(C, 9))
    dwv = np.ascontiguousarray(Wdw[2 * C:, 0].reshape(C, 9))
    dwq_b, dwk_b, dwv_b = (a.astype(bf16) for a in (dwq, dwk, dwv))

    dwdiag = np.zeros((3, NPE, 3, 128, 128), f32)
    for ti, dwt in enumerate((dwq, dwk, dwv)):
        for i, t in enumerate(PE_TAPS[ti]):
            for pt in range(3):
                dwdiag[ti, i, pt][np.arange(128), np.arange(128)] = \
                    dwt[128 * pt:128 * pt + 128, t]
    dwdiag = dwdiag.astype(bf16)

    blk = np.zeros((96, 4 * 96), f32)
    for p in range(4):
        blk[0:48, 96 * p:96 * p + 48] = 1.0
        blk[48:96, 96 * p + 48:96 * p + 96] = 1.0
    blkmask = blk.astype(bf16)
    tempT = np.repeat(temperature.reshape(HEADS), C // HEADS).reshape(1, C)
    tempT = tempT.astype(f32)

    in_maps = []
    for c in range(8):
        b, half = c // 2, c % 2
        r0 = half * HALF

        def shard(t):
            s = np.zeros((C, HALF + 2, W_), f32)
            s[:, 1:HALF + 1] = t[b, :, r0:r0 + HALF]
            if r0 > 0:
                s[:, 0] = t[b, :, r0 - 1]
            if r0 + HALF < H:
                s[:, HALF + 1] = t[b, :, r0 + HALF]
            return s.astype(bf16)

        in_maps.append({
            "x": shard(x), "y": shard(y),
            "wqT": wqT, "wkT": wkT, "wvT": wvT, "wpT": wpT,
            "dwq": dwq_b, "dwk": dwk_b, "dwv": dwv_b,
            "dwdiag": dwdiag, "blkmask": blkmask,
            "tempT": tempT,
        })

    import os
    kw = {}
    if os.environ.get("KBENCH_TRACE"):
        kw = dict(trace=True)
    res = run_bass_kernel_spmd(nc, in_maps, list(range(8)), **kw)
    kernel._last_result = res

    out = np.zeros((B, C, H, W_), f32)
    for c in range(8):
        b, half = c // 2, c % 2
        out[b, :, half * HALF:(half + 1) * HALF] = \
            np.asarray(res.results[c]["out"], f32).reshape(C, HALF, W_)
    return out


# revision 27
# speedup vs baseline: 1.2458x; 1.2073x over previous
"""Trainium2 Bass kernel for nn_CrossAttention (dense_transformer).  v3

Sharding: 8 cores = (batch 0..3) x (image half 0..1), 64 rows + 1 halo row
each side per core.  Only per-head Gram matrices and l2-norm square sums
are AllGather'd between the two cores of a batch.

v3 structure (vs v2, from trace analysis: DVE 69%/PE 65%/ACT 56% busy):
  - conv1x1 outputs land in a 130-pitch padded SBUF layout A (2 zero pad
    cols between rows, row base 4B-aligned), conv psum f32 N=512.
  - depthwise 9 taps split: 2 taps (q,k) / 4 taps (v) on PE as chained
    diagonal matmuls into bf16 psum (alignment-free); the ACT psum->acc
    copy doubles as the accumulator init; remaining taps are single-pass
    DVE scalar_tensor_tensor (2x mode): dx==0 taps read A at even offsets,
    odd taps read S = A<<1 (built by DMA, SBUF->SBUF) at even offsets.
  - acc (dw output) is padless pitch-128: flat DVE out APs, flat norms,
    flat proj windows, and batched DMA-xbar transposes ([128, 1024] ->
    [128, 8, 128] per-block) straight from acc -- no PE transposes.
  - Gram accumulated per spatial row over transposed q/k tiles (64 rows
    x 4 pair-blocks); q,k processed in interleaved 32-row half-tensors.
  - attention is folded into Wproj: M^T = A_head^T-weighted WprojT built
    on-PE after softmax; final conv1x1 uses M^T directly on the stored
    depthwise-v (attn@v matmuls and their copies disappear).
"""
import numpy as np
import ml_dtypes

import concourse.bass as bass
import concourse.bacc as bacc
from concourse import mybir
from concourse.bass_utils import run_bass_kernel_spmd
from concourse.tile import TileContext

F32 = mybir.dt.float32
BF16 = mybir.dt.bfloat16
ALU = mybir.AluOpType
AF = mybir.ActivationFunctionType

C = 384
HEADS = 8
W = 128
HALF = 64
HR = 32                  # output rows per half-tensor pass
AR = HR + 2              # conv rows per half (1 halo each side)
PITCH = 130              # A row pitch: [pad, pad, 128 data]
ASZ = 2 + AR * PITCH + 6   # 4428; leading/trailing zero guards
ACC = HR * W             # 4096, padless acc per half

# tap t: dy = t//3-1, dx = t%3-1; base offset into A for out (row j, col c)
#   A[4 + 130*(j+1+dy) + c + dx]
def _ab(t):
    dy, dx = t // 3 - 1, t % 3 - 1
    return 4 + PITCH * (1 + dy) + dx

# per-tensor tap split (tensor 0=q, 1=k, 2=v)
PE_TAPS = ((0, 2, 6, 8),) * 3
DVE_A_TAPS = ((1, 4, 7),) * 3                       # even offsets on A
DVE_S_TAPS = ((3, 5),) * 3                          # odd: even offsets on S
NPE = max(len(p) for p in PE_TAPS)
FDC = 2048                                          # DVE chunk (TT 2x sweet)

DWWIN = [(4 * w, 4) for w in range(HR // 4)]                         # 8 wins
CVWIN = [(4 * w, min(4, AR - 4 * w)) for w in range((AR + 3) // 4)]  # 9 wins
PJWIN = [(4 * w, 4) for w in range(HALF // 4)]                       # 16 wins


def _build():
    nc = bacc.Bacc(num_devices=8)

    x_ext = nc.declare_dram_parameter("x", [C, HALF + 2, W], BF16, isOutput=False)
    y_ext = nc.declare_dram_parameter("y", [C, HALF + 2, W], BF16, isOutput=False)
    wqT_ext = nc.declare_dram_parameter("wqT", [C, C], BF16, isOutput=False)
    wkT_ext = nc.declare_dram_parameter("wkT", [C, C], BF16, isOutput=False)
    wvT_ext = nc.declare_dram_parameter("wvT", [C, C], BF16, isOutput=False)
    wpT_ext = nc.declare_dram_parameter("wpT", [4, 96, C], BF16, isOutput=False)
    dwq_ext = nc.declare_dram_parameter("dwq", [C, 9], F32, isOutput=False)
    dwk_ext = nc.declare_dram_parameter("dwk", [C, 9], F32, isOutput=False)
    dwv_ext = nc.declare_dram_parameter("dwv", [C, 9], F32, isOutput=False)
    dg_ext = nc.declare_dram_parameter("dwdiag", [3, NPE, 3, 128, 128],
                                       BF16, isOutput=False)
    mask_ext = nc.declare_dram_parameter("blkmask", [96, 4 * 96], BF16,
                                         isOutput=False)
    tT_ext = nc.declare_dram_parameter("tempT", [1, C], F32, isOutput=False)
    out_ext = nc.declare_dram_parameter("out", [C, HALF * W], BF16, isOutput=True)

    SLEN = 96 * 4 * 96
    CCN = SLEN + 2 * C
    cc_in = nc.dram_tensor("cc_in", [1, CCN], F32)
    cc_out = nc.dram_tensor("cc_out", [2, CCN], F32)
    rn_scr = nc.dram_tensor("rn_scr", [2, C], F32)

    with TileContext(nc) as tc:
        with tc.tile_pool(name="const", bufs=1) as cpool:
            w_sb = []   # [tensor][kt] -> [128, C]
            for ti, ext in enumerate((wqT_ext, wkT_ext, wvT_ext)):
                tiles = []
                for kt in range(3):
                    t_ = cpool.tile([128, C], BF16, tag=f"w{ti}{kt}", name=f"w{ti}{kt}")
                    nc.scalar.dma_start(out=t_[:], in_=ext[128 * kt:128 * kt + 128, :])
                    tiles.append(t_)
                w_sb.append(tiles)
            wp_sb = [cpool.tile([96, C], BF16, tag=f"wp{g}", name=f"wp{g}")
                     for g in range(4)]
            for g in range(4):
                nc.scalar.dma_start(out=wp_sb[g][:], in_=wpT_ext[g])
            dw_sb = []  # [tensor][pt] -> [128, 9] f32
            for ti, ext in enumerate((dwq_ext, dwk_ext, dwv_ext)):
                tiles = []
                for pt in range(3):
                    t_ = cpool.tile([128, 9], F32, tag=f"dw{ti}{pt}", name=f"dw{ti}{pt}")
                    nc.scalar.dma_start(out=t_[:],
                                        in_=ext[128 * pt:128 * pt + 128, :])
                    tiles.append(t_)
                dw_sb.append(tiles)
            dg_sb = [[[cpool.tile([128, 128], BF16, tag=f"dg{ti}{i}{pt}",
                                  name=f"dg{ti}{i}{pt}")
                       for pt in range(3)]
                      for i in range(len(PE_TAPS[ti]))] for ti in range(3)]
            for ti in range(3):
                for i in range(len(PE_TAPS[ti])):
                    for pt in range(3):
                        nc.scalar.dma_start(out=dg_sb[ti][i][pt][:],
                                            in_=dg_ext[ti, i, pt])
            mask_sb = cpool.tile([96, 4 * 96], BF16, tag="mask", name="mask")
            nc.scalar.dma_start(out=mask_sb[:], in_=mask_ext[:])
            tT_sb = cpool.tile([1, C], F32, tag="tempT", name="tempT")
            nc.scalar.dma_start(out=tT_sb[:], in_=tT_ext[:])

            nsq_q = cpool.tile([128, 6], F32, tag="nsqq", name="nsqq")
            nsq_k = cpool.tile([128, 6], F32, tag="nsqk", name="nsqk")
            sp_sb = cpool.tile([96, 4 * 96], F32, tag="spsb", name="spsb")
            mT_sb = [cpool.tile([128, C], BF16, tag=f"mT{kt}", name=f"mT{kt}")
                     for kt in range(3)]

            # ---------------- per-half-tensor worker ----------------
            def half_pass(pool, psp, ti, src_ext, h, accs, acc_off, nsq):
                """conv1x1 + depthwise for rows [32h, 32h+32).
                accs[pt]: tile AP target for dw output (pitch-128);
                acc_off: element offset of this half inside accs[pt]."""
                src = []
                for kt in range(3):
                    s_ = pool.tile([128, AR * W], BF16, tag=f"src{kt}",
                                   name=f"src{kt}", bufs=1)
                    nc.sync.dma_start(
                        out=s_[:], in_=src_ext[128 * kt:128 * kt + 128,
                                               HR * h:HR * h + AR, :])
                    src.append(s_)
                As, Ss = [], []
                for pt in range(3):
                    A = pool.tile([128, ASZ], BF16, tag=f"A{pt}", name=f"A{pt}")
                    A3 = A[:, 2:2 + AR * PITCH].rearrange(
                        "p (r c) -> p r c", c=PITCH)
                    nc.vector.memset(A3[:, :, 0:2], 0.0)
                    nc.vector.memset(A[:, 0:2], 0.0)
                    nc.vector.memset(A[:, 2 + AR * PITCH:], 0.0)
                    for w0, rw in CVWIN:
                        cps = psp.tile([128, 512], F32, tag="convps", name="convps")
                        for kt in range(3):
                            nc.tensor.matmul(
                                cps[:, 0:rw * W],
                                w_sb[ti][kt][:, 128 * pt:128 * pt + 128],
                                src[kt][:, w0 * W:(w0 + rw) * W],
                                start=(kt == 0), stop=(kt == 2))
                        nc.scalar.copy(
                            A3[:, w0:w0 + rw, 2:2 + W],
                            cps[:, 0:rw * W].rearrange("p (r c) -> p r c", c=W))
                    S = pool.tile([128, ASZ], BF16, tag=f"S{pt}", name=f"S{pt}",
                                  bufs=1)
                    nc.scalar.dma_start(out=S[:, 0:ASZ - 1], in_=A[:, 1:ASZ])
                    As.append(A)
                    Ss.append(S)
                for pt in range(3):
                    A = As[pt]
                    acc = accs[pt]
                    for j0, rw in DWWIN:
                        dps = psp.tile([128, 512], F32, tag="dwps", name="dwps")
                        pe = PE_TAPS[ti]
                        for i, t in enumerate(pe):
                            b = _ab(t) + PITCH * j0
                            nc.tensor.matmul(
                                dps[:, 0:rw * W],
                                dg_sb[ti][i][pt][:],
                                A[:, b:b + rw * PITCH].rearrange(
                                    "p (r c) -> p r c", c=PITCH)[:, :, 0:W],
                                start=(i == 0), stop=(i == len(pe) - 1))
                        nc.scalar.copy(
                            acc[:, acc_off + j0 * W:acc_off + (j0 + rw) * W],
                            dps[:, 0:rw * W])
                # DVE taps: TS (4x) + TT (2x) pairs, FDC chunks, pt-interleaved
                # (STT runs 1x on this DVE ucode; pt rotation hides drain)
                for ck in range(ACC // FDC):
                    rr = FDC // W   # rows per chunk
                    r0 = ck * rr
                    for t in DVE_A_TAPS[ti] + DVE_S_TAPS[ti]:
                        odd = t in DVE_S_TAPS[ti]
                        for pt in range(3):
                            src = Ss[pt] if odd else As[pt]
                            b = (_ab(t) - 1 if odd else _ab(t)) + PITCH * r0
                            tmp = pool.tile([128, FDC], BF16, tag="tmp",
                                            name="tmp", bufs=2)
                            nc.vector.tensor_scalar(
                                tmp[:].rearrange("p (r c) -> p r c", c=W),
                                src[:, b:b + rr * PITCH].rearrange(
                                    "p (r c) -> p r c", c=PITCH)[:, :, 0:W],
                                dw_sb[ti][pt][:, t:t + 1], None, ALU.mult)
                            o0 = acc_off + ck * FDC
                            nc.vector.tensor_tensor(
                                out=accs[pt][:, o0:o0 + FDC], in0=tmp[:],
                                in1=accs[pt][:, o0:o0 + FDC], op=ALU.add)
                if nsq is not None:
                    for pt in range(3):
                        # S is dead after the taps; reuse as Square scratch
                        nc.scalar.activation(
                            Ss[pt][:, 0:ACC],
                            accs[pt][:, acc_off:acc_off + ACC], AF.Square,
                            accum_out=nsq[:, 2 * pt + h:2 * pt + h + 1])

            # ================== q/k phase with Gram ==================
            with tc.tile_pool(name="qk", bufs=2) as pool, \
                 tc.tile_pool(name="ps1", bufs=2, space="PSUM") as psp, \
                 tc.tile_pool(name="gramp", bufs=1, space="PSUM") as gramp:
                s_ps = gramp.tile([96, 4, 96], F32, tag="sps", name="sps")
                for h in range(2):
                    acck = [pool.tile([128, ACC], BF16, tag=f"acc{pt}",
                                      name=f"kacc{pt}") for pt in range(3)]
                    half_pass(pool, psp, 1, y_ext, h, acck, 0, nsq_k)
                    accq = [pool.tile([128, ACC], BF16, tag=f"acc{pt}",
                                      name=f"qacc{pt}") for pt in range(3)]
                    half_pass(pool, psp, 0, x_ext, h, accq, 0, nsq_q)
                    for g in range(4):
                        kT = pool.tile([128, 8, C], BF16, tag="kT", name="kT",
                                       bufs=1)
                        qT = pool.tile([128, 8, C], BF16, tag="qT", name="qT",
                                       bufs=1)
                        for pt in range(3):
                            nc.sync.dma_start_transpose(
                                kT[:, :, 128 * pt:128 * pt + 128],
                                acck[pt][:, 1024 * g:1024 * g + 1024])
                            nc.sync.dma_start_transpose(
                                qT[:, :, 128 * pt:128 * pt + 128],
                                accq[pt][:, 1024 * g:1024 * g + 1024])
                        for j in range(8):
                            r = HR * h + 8 * g + j
                            for p in range(4):
                                nc.tensor.matmul(
                                    s_ps[:, p, :],
                                    qT[:, j, 96 * p:96 * p + 96],
                                    kT[:, j, 96 * p:96 * p + 96],
                                    start=(r == 0), stop=(r == HALF - 1),
                                    skip_group_check=True)
                nc.scalar.copy(
                    sp_sb[:].rearrange("p (g n) -> p g n", n=96), s_ps[:])

            # ============ v phase + collective + softmax + proj ============
            nsqr_q = cpool.tile([128, 3], F32, tag="nsqrq", name="nsqrq")
            nsqr_k = cpool.tile([128, 3], F32, tag="nsqrk", name="nsqrk")
            nc.vector.tensor_tensor(
                out=nsqr_q[:],
                in0=nsq_q[:].rearrange("p (t h) -> p t h", h=2)[:, :, 0],
                in1=nsq_q[:].rearrange("p (t h) -> p t h", h=2)[:, :, 1],
                op=ALU.add)
            nc.vector.tensor_tensor(
                out=nsqr_k[:],
                in0=nsq_k[:].rearrange("p (t h) -> p t h", h=2)[:, :, 0],
                in1=nsq_k[:].rearrange("p (t h) -> p t h", h=2)[:, :, 1],
                op=ALU.add)

            ccs = [cpool.tile([96, 4 * 96], F32, tag=f"ccs{r}", name=f"ccs{r}")
                   for r in range(2)]
            ccnq = [cpool.tile([128, 3], F32, tag=f"ccnq{r}", name=f"ccnq{r}")
                    for r in range(2)]
            ccnk = [cpool.tile([128, 3], F32, tag=f"ccnk{r}", name=f"ccnk{r}")
                    for r in range(2)]

            with tc.tile_pool(name="vp", bufs=2) as pool, \
                 tc.tile_pool(name="ps2", bufs=2, space="PSUM") as psp, \
                 tc.tile_pool(name="vstp", bufs=1) as vstp, \
                 tc.tile_pool(name="smp", bufs=1) as smp:
                vst = [vstp.tile([128, HALF * W], BF16, tag=f"vst{pt}",
                                 name=f"vst{pt}") for pt in range(3)]
                for h in range(2):
                    half_pass(pool, psp, 2, y_ext, h, vst, ACC * h, None)

                # ---- collective (gpsimd-only critical section) ----
                with tc.tile_critical():
                    ccsem = nc.alloc_semaphore("ccsem")
                    sv = 0
                    nc.gpsimd.dma_start(
                        out=cc_in[0, 0:SLEN].rearrange("(p n) -> p n", p=96),
                        in_=sp_sb[:]).then_inc(ccsem, 16)
                    sv += 16
                    nc.gpsimd.dma_start(
                        out=cc_in[0, SLEN:SLEN + C].rearrange(
                            "(n p) -> p n", p=128),
                        in_=nsqr_q[:]).then_inc(ccsem, 16)
                    sv += 16
                    nc.gpsimd.dma_start(
                        out=cc_in[0, SLEN + C:].rearrange(
                            "(n p) -> p n", p=128),
                        in_=nsqr_k[:]).then_inc(ccsem, 16)
                    sv += 16
                    nc.gpsimd.wait_ge(ccsem, sv)
                    nc.gpsimd.collective_compute(
                        "AllGather", ALU.bypass,
                        replica_groups=[[0, 1], [2, 3], [4, 5], [6, 7]],
                        ins=[cc_in[:].opt()],
                        outs=[cc_out[:].opt()],
                    ).then_inc(ccsem, 1)
                    sv += 1
                    nc.gpsimd.wait_ge(ccsem, sv)
                    for r in range(2):
                        nc.gpsimd.dma_start(
                            out=ccs[r][:],
                            in_=cc_out[r, 0:SLEN].rearrange(
                                "(p n) -> p n", p=96)).then_inc(ccsem, 16)
                        sv += 16
                        nc.gpsimd.dma_start(
                            out=ccnq[r][:],
                            in_=cc_out[r, SLEN:SLEN + C].rearrange(
                                "(n p) -> p n", p=128)).then_inc(ccsem, 16)
                        sv += 16
                        nc.gpsimd.dma_start(
                            out=ccnk[r][:],
                            in_=cc_out[r, SLEN + C:].rearrange(
                                "(n p) -> p n", p=128)).then_inc(ccsem, 16)
                        sv += 16
                    nc.gpsimd.wait_ge(ccsem, sv)

                # -------------------- softmax --------------------
                s_full = smp.tile([96, 4, 96], F32, tag="sfull", name="sfull")
                nc.vector.tensor_tensor(
                    out=s_full[:],
                    in0=ccs[0][:].rearrange("p (g n) -> p g n", n=96),
                    in1=ccs[1][:].rearrange("p (g n) -> p g n", n=96),
                    op=ALU.add)
                rnq = smp.tile([128, 3], F32, tag="rnq", name="rnq")
                rnk = smp.tile([128, 3], F32, tag="rnk", name="rnk")
                nc.vector.tensor_tensor(out=rnq[:], in0=ccnq[0][:],
                                        in1=ccnq[1][:], op=ALU.add)
                nc.vector.tensor_tensor(out=rnk[:], in0=ccnk[0][:],
                                        in1=ccnk[1][:], op=ALU.add)
                nc.scalar.activation(rnq[:], rnq[:], AF.Sqrt)
                nc.scalar.activation(rnk[:], rnk[:], AF.Sqrt)
                nc.vector.tensor_scalar_max(rnq[:], rnq[:], 1e-12)
                nc.vector.tensor_scalar_max(rnk[:], rnk[:], 1e-12)
                nc.vector.reciprocal(rnq[:], rnq[:])
                nc.vector.reciprocal(rnk[:], rnk[:])

                rnqT = smp.tile([1, C], F32, tag="rnqT", name="rnqT")
                rnkT = smp.tile([1, C], F32, tag="rnkT", name="rnkT")
                with tc.tile_critical():
                    rsem = nc.alloc_semaphore("rsem")
                    nc.gpsimd.dma_start(
                        out=rn_scr[0, :].rearrange("(n p) -> p n", p=128),
                        in_=rnq[:]).then_inc(rsem, 16)
                    nc.gpsimd.dma_start(
                        out=rn_scr[1, :].rearrange("(n p) -> p n", p=128),
                        in_=rnk[:]).then_inc(rsem, 16)
                    nc.gpsimd.wait_ge(rsem, 32)
                    nc.gpsimd.dma_start(
                        out=rnqT[:], in_=rn_scr[0:1, :]).then_inc(rsem, 16)
                    nc.gpsimd.dma_start(
                        out=rnkT[:], in_=rn_scr[1:2, :]).then_inc(rsem, 16)
                    nc.gpsimd.wait_ge(rsem, 64)
                nc.vector.tensor_tensor(out=rnqT[:], in0=rnqT[:],
                                        in1=tT_sb[:], op=ALU.mult)

                outer_ps = psp.tile([96, 4, 96], F32, tag="outerps",
                                    name="outerps", bufs=1)
                for p in range(4):
                    nc.tensor.matmul(
                        outer_ps[:, p, :],
                        rnqT[0:1, 96 * p:96 * p + 96],
                        rnkT[0:1, 96 * p:96 * p + 96],
                        start=True, stop=True)
                logits = smp.tile([96, 4, 96], F32, tag="logits", name="logits")
                nc.vector.tensor_tensor(out=logits[:], in0=s_full[:],
                                        in1=outer_ps[:], op=ALU.mult)
                expv = smp.tile([96, 4 * 96], F32, tag="expv", name="expv")
                nc.scalar.activation(
                    expv[:], logits[:].rearrange("p g n -> p (g n)"), AF.Exp)
                expm = smp.tile([96, 4, 96], F32, tag="expm", name="expm")
                nc.vector.tensor_tensor(
                    out=expm[:],
                    in0=expv[:].rearrange("p (g n) -> p g n", n=96),
                    in1=mask_sb[:].rearrange("p (g n) -> p g n", n=96),
                    op=ALU.mult)
                rs = smp.tile([96, 4], F32, tag="rs", name="rs")
                nc.vector.tensor_reduce(out=rs[:], in_=expm[:],
                                        axis=mybir.AxisListType.X, op=ALU.add)
                nc.vector.reciprocal(rs[:], rs[:])
                attn = smp.tile([96, 4, 96], BF16, tag="attn", name="attn")
                for p in range(4):
                    nc.vector.tensor_scalar(
                        attn[:, p, :], expm[:, p, :], rs[:, p:p + 1],
                        None, ALU.mult)

                # ---- fold attention into Wproj: mT[cd, o] ----
                for g in range(4):
                    # attn's off-diagonal 48x48 blocks are zero (masked), so
                    # one 96x96 matmul per pair-block is exact.
                    mps = psp.tile([96, C], F32, tag="mps", name="mps", bufs=1)
                    nc.tensor.matmul(
                        mps[:], attn[:, g, :], wp_sb[g][:],
                        start=True, stop=True)
                    for q in range(3):  # 32-part pieces (PSUM window rule)
                        r0 = 96 * g + 32 * q
                        nc.scalar.copy(
                            mT_sb[r0 // 128][r0 % 128:r0 % 128 + 32, :],
                            mps[32 * q:32 * q + 32, :])

                # ------------- proj: out = mT^T @ vdw -------------
                for ot in range(3):
                    for j0, rw in PJWIN:
                        pps = psp.tile([128, 512], F32, tag="projps",
                                       name="pps")
                        for kt in range(3):
                            nc.tensor.matmul(
                                pps[:, 0:rw * W],
                                mT_sb[kt][:, 128 * ot:128 * ot + 128],
                                vst[kt][:, j0 * W:(j0 + rw) * W],
                                start=(kt == 0), stop=(kt == 2))
                        osb = pool.tile([128, 512], BF16, tag="osb", name="osb")
                        nc.scalar.copy(osb[:, 0:rw * W], pps[:, 0:rw * W])
                        nc.sync.dma_start(
                            out=out_ext[128 * ot:128 * ot + 128,
                                        j0 * W:(j0 + rw) * W],
                            in_=osb[:, 0:rw * W])
    return nc


_BUILD_CACHE = {}


def _get_program():
    if "nc" not in _BUILD_CACHE:
        nc = _build()
        if not nc.is_finalized():
            nc.finalize()
        _BUILD_CACHE["nc"] = nc
    return _BUILD_CACHE["nc"]


def kernel(x, y, Wq, Wkv, Wdw, Wproj, temperature):
    B, C_, H, W_ = x.shape
    assert C_ == C and W_ == W and H == 2 * HALF
    nc = _get_program()

    f32 = np.float32
    bf16 = ml_dtypes.bfloat16
    x = np.asarray(x, f32)
    y = np.asarray(y, f32)
    Wq = np.asarray(Wq, f32)
    Wkv = np.asarray(Wkv, f32)
    Wdw = np.asarray(Wdw, f32)
    Wproj = np.asarray(Wproj, f32)
    temperature = np.asarray(temperature, f32)

    wqT = np.ascontiguousarray(Wq.T).astype(bf16)
    wkT = np.ascontiguousarray(Wkv[:C].T).astype(bf16)
    wvT = np.ascontiguousarray(Wkv[C:].T).astype(bf16)
    wpT = np.ascontiguousarray(Wproj.T.reshape(4, 96, C)).astype(bf16)
    dwq = np.ascontiguousarray(Wdw[0:C, 0].reshape(C, 9))
    dwk = np.ascontiguousarray(Wdw[C:2 * C, 0].reshape(C, 9))
    dwv = np.ascontiguousarray(Wdw[2 * C:, 0].reshape(C, 9))

    dwdiag = np.zeros((3, NPE, 3, 128, 128), f32)
    for ti, dwt in enumerate((dwq, dwk, dwv)):
        for i, t in enumerate(PE_TAPS[ti]):
            for pt in range(3):
                dwdiag[ti, i, pt][np.arange(128), np.arange(128)] = \
                    dwt[128 * pt:128 * pt + 128, t]
    dwdiag = dwdiag.astype(bf16)

    blk = np.zeros((96, 4 * 96), f32)
    for p in range(4):
        blk[0:48, 96 * p:96 * p + 48] = 1.0
        blk[48:96, 96 * p + 48:96 * p + 96] = 1.0
    blkmask = blk.astype(bf16)
    tempT = np.repeat(temperature.reshape(HEADS), C // HEADS).reshape(1, C)
    tempT = tempT.astype(f32)

    in_maps = []
    for c in range(8):
        b, half = c // 2, c % 2
        r0 = half * HALF

        def shard(t):
            s = np.zeros((C, HALF + 2, W_), f32)
            s[:, 1:HALF + 1] = t[b, :, r0:r0 + HALF]
            if r0 > 0:
                s[:, 0] = t[b, :, r0 - 1]
            if r0 + HALF < H:
                s[:, HALF + 1] = t[b, :, r0 + HALF]
            return s.astype(bf16)

        in_maps.append({
            "x": shard(x), "y": shard(y),
            "wqT": wqT, "wkT": wkT, "wvT": wvT, "wpT": wpT,
            "dwq": dwq, "dwk": dwk, "dwv": dwv,
            "dwdiag": dwdiag, "blkmask": blkmask,
            "tempT": tempT,
        })

    import os
    kw = {}
    if os.environ.get("KBENCH_TRACE"):
        kw = dict(trace=True)
    res = run_bass_kernel_spmd(nc, in_maps, list(range(8)), **kw)
    kernel._last_result = res

    out = np.zeros((B, C, H, W_), f32)
    for c in range(8):
        b, half = c // 2, c % 2
        out[b, :, half * HALF:(half + 1) * HALF] = \
            np.asarray(res.results[c]["out"], f32).reshape(C, HALF, W_)
    return out
